# revision 43
# baseline (speedup 1.0000x reference)
"""GAT message-passing model on 8 Trainium2 NeuronCores.

Strategy: edges sorted by destination on the host; nodes split into 8
contiguous ranges balanced by incoming-edge count (one per core).  Windows of
<=128 contiguous dst nodes with <=TPW*128 edges, padded to TPW tiles of 128
edge slots so all 8 cores run one identical SPMD instruction stream.

The host pre-gathers (pure layout work, no arithmetic) the transposed node
features for every edge slot and window-node block, plus both one-hot
orientations of the edge->node incidence.  The device kernel is then a single
homogeneous window loop with no tables, no fences and no indirect gathers
except the final y scatter:

  per window:  qwin = nfTwin^T @ Wq (PE) -> SBUF (ACT copy)
  per tile:    kke[e,hd] = nfT_e^T Wk + efT^T We   (PE -> psum, ACT copy)
               qe[e,hd]  = ohT^T @ qwin            (PE, same psum tile)
               prod      = qe * kke                (DVE, one PSUM read)
               logit     = reduce_d(prod)          (DVE)
               w         = exp(logit)              (ACT, into wv_ext[:,512:])
               v[e,hd]   = nfT_e^T @ Wv            (PE; ACT copy to SBUF)
               wv        = w (bcast) * v           (POOL; every 8th on DVE)
               agg, den += ohE^T @ [wv | w]        (PE segment-sum, deferred
                                                    one tile for pipelining)
  finalize:    den+eps, recip (DVE); u=relu(agg)*wd (DVE STT);
               z=sum(u*recip) (DVE STT+accum); y=1/(1+exp(-(z+bd)))
               (ACT exp + DVE); dense per-window y DMA, host scatters.
"""

import numpy as np
import ml_dtypes

import concourse.bass as bass
import concourse.bacc as bacc
import concourse.mybir as mybir
import concourse.tile as tile

BF16 = ml_dtypes.bfloat16
FP8 = ml_dtypes.float8_e4m3

H, DH = 8, 64
DIN, DE = 256, 64
DOUT = H * DH  # 512
N_CORES = 8
TPW = 8  # edge tiles per window
K_FP8 = False
V_FP8 = False


# ----------------------------------------------------------------------------
# Host-side planning (layout only -- no arithmetic on features/weights)
# ----------------------------------------------------------------------------

def make_plan(src, dst, n_nodes, n_cores, tpw):
    E = src.shape[0]
    perm = np.argsort(dst, kind="stable")
    s_src = src[perm]
    s_dst = dst[perm]
    deg = np.bincount(dst, minlength=n_nodes)
    cum = np.concatenate([[0], np.cumsum(deg)])

    cuts = [0]
    for c in range(1, n_cores):
        target = c * E / n_cores
        n = int(np.searchsorted(cum, target))
        n = max(cuts[-1] + 1, min(n, n_nodes - (n_cores - c)))
        cuts.append(n)
    cuts.append(n_nodes)

    cores = []
    for c in range(n_cores):
        nlo, nhi = cuts[c], cuts[c + 1]
        wins = []
        n = nlo
        while n < nhi:
            n2 = n
            edges = 0
            while n2 < nhi and (n2 - n) < 128:
                if edges + deg[n2] > tpw * 128:
                    break
                edges += deg[n2]
                n2 += 1
            assert n2 > n, f"node {n} degree {deg[n]} > {tpw*128}"
            wins.append((n, n2))
            n = n2
        cores.append(dict(nlo=nlo, nhi=nhi, wins=wins))

    NWIN = max(len(c["wins"]) for c in cores)
    LMAX = max(c["nhi"] - c["nlo"] for c in cores)
    return dict(cores=cores, NWIN=NWIN, LMAX=LMAX, TPW=tpw,
                s_src=s_src, s_dst=s_dst, perm=perm, cum=cum)


def make_core_inputs(plan, core_idx, nf_bf, nf_e_dt, ef_sorted_bf):
    """Per-core pre-gathered tensors.  nf_e_dt: nf in the k-path dtype."""
    tpw = plan["TPW"]
    NWIN = plan["NWIN"]
    LMAX = plan["LMAX"]
    core = plan["cores"][core_idx]
    s_src, s_dst, cum = plan["s_src"], plan["s_dst"], plan["cum"]
    nlo = core["nlo"]
    L = core["nhi"] - nlo
    trash = LMAX

    nfe = np.zeros((NWIN, 128, tpw, 2, 128), nf_e_dt.dtype)
    ohT = np.zeros((NWIN, 128, tpw * 128), BF16)
    ohE = np.zeros((NWIN, 128, tpw * 128), BF16)
    efT = np.zeros((NWIN, 64, tpw * 128), BF16)
    nfw = np.zeros((NWIN, 128, 2, 128), BF16)
    wnodes = np.full((NWIN, 128, 1), trash, np.int32)

    for w, (wn_lo, wn_hi) in enumerate(core["wins"]):
        e0, e1 = cum[wn_lo], cum[wn_hi]
        cnt = e1 - e0
        Lw = wn_hi - wn_lo
        wnodes[w, :Lw, 0] = np.arange(wn_lo, wn_hi) - nlo
        sl = np.arange(cnt)
        t_idx = sl // 128
        p_idx = sl % 128
        dl = s_dst[e0:e1] - wn_lo
        # transposed gathered node features: nfe[w, p, t, i, e] = nf[src, i*128+p]
        blk = nf_e_dt[s_src[e0:e1]].reshape(cnt, 2, 128)  # [slot, i, p]
        nfe[w][:, t_idx, :, p_idx] = blk.transpose(0, 2, 1)
        # one-hots
        ohT[w][dl, t_idx * 128 + p_idx] = 1.0
        ohE[w][p_idx, t_idx * 128 + dl] = 1.0
        # transposed edge features
        efT[w][:, t_idx * 128 + p_idx] = ef_sorted_bf[e0:e1].T
        # transposed window-node features for q
        nblk = nf_bf[wn_lo:wn_hi].reshape(Lw, 2, 128)  # [nl, i, p]
        nfw[w][:, :, :Lw] = nblk.transpose(2, 1, 0)
    out = dict(nfe=nfe.reshape(NWIN, 128, tpw * 256),
               ohT=ohT, ohE=ohE, efT=efT,
               nfw=nfw.reshape(NWIN, 128, 256),
               wnodes=wnodes, L=L, nlo=nlo)
    if K_FP8 != V_FP8:
        out["nfe2"] = out["nfe"].astype(FP8 if V_FP8 else BF16)
    return out


def make_global_inputs(Wq, Wk, Wv, We, Wd):
    scale = 1.0 / np.sqrt(DH)
    # DoubleRow K-packing: w3[p, i, n] = W[i*128+p, n]
    wk3 = np.zeros((128, 2, DOUT), FP8 if K_FP8 else BF16)
    wv3 = np.zeros((128, 2, DOUT), FP8 if V_FP8 else BF16)
    for i in range(2):
        wk3[:, i, :] = Wk[i * 128:(i + 1) * 128].astype(wk3.dtype)
        wv3[:, i, :] = Wv[i * 128:(i + 1) * 128].astype(wv3.dtype)
    # wq[p, c*512+n] = (Wq*scale)[c*128+p, n]
    wq = np.concatenate([(Wq * scale)[:128], (Wq * scale)[128:256]],
                        axis=1).astype(BF16)
    we = We.astype(BF16)                       # [64, 512]
    wdrow = np.tile(Wd.reshape(1, DOUT), (128, 1)).astype(BF16)
    return dict(wk3=wk3.reshape(128, 2 * DOUT), wv3=wv3.reshape(128, 2 * DOUT),
                wq=wq, we=we, wdrow=wdrow)


# ----------------------------------------------------------------------------
# Device kernel emission (identical instruction stream on every core)
# ----------------------------------------------------------------------------

def build_nc(NWIN, tpw, LMAX, bd0, k_fp8, v_fp8, dbg=False):
    dt = mybir.dt
    bf16, f32, i32 = dt.bfloat16, dt.float32, dt.int32
    kdt = dt.float8e4 if k_fp8 else dt.bfloat16
    vdt = dt.float8e4 if v_fp8 else dt.bfloat16
    YROWS = LMAX + 128

    nc = bacc.Bacc("TRN2", target_bir_lowering=False, debug=False)

    t_nfe = nc.dram_tensor("nfe", [NWIN, 128, tpw * 256], kdt, kind="ExternalInput")
    t_nfe2 = (nc.dram_tensor("nfe2", [NWIN, 128, tpw * 256], vdt,
                             kind="ExternalInput") if k_fp8 != v_fp8 else None)
    t_ohT = nc.dram_tensor("ohT", [NWIN, 128, tpw * 128], bf16, kind="ExternalInput")
    t_ohE = nc.dram_tensor("ohE", [NWIN, 128, tpw * 128], bf16, kind="ExternalInput")
    t_efT = nc.dram_tensor("efT", [NWIN, 64, tpw * 128], bf16, kind="ExternalInput")
    t_nfw = nc.dram_tensor("nfw", [NWIN, 128, 256], bf16, kind="ExternalInput")
    t_wk3 = nc.dram_tensor("wk3", [128, 2 * DOUT], kdt, kind="ExternalInput")
    t_wv3 = nc.dram_tensor("wv3", [128, 2 * DOUT], vdt, kind="ExternalInput")
    t_wq = nc.dram_tensor("wq", [128, 2 * DOUT], bf16, kind="ExternalInput")
    t_we = nc.dram_tensor("we", [64, DOUT], bf16, kind="ExternalInput")
    t_wdrow = nc.dram_tensor("wdrow", [128, DOUT], bf16, kind="ExternalInput")

    t_y = nc.dram_tensor("y_out", [NWIN, 128, 1], f32, kind="ExternalOutput")

    MM = mybir.MatmulPerfMode.DoubleRow

    with tile.TileContext(nc, pool_alloc_mode="queue") as tc:
        with tc.tile_pool(name="wt", bufs=1) as wt, \
             tc.tile_pool(name="win", bufs=4) as win, \
             tc.tile_pool(name="tb", bufs=5) as tb, \
             tc.tile_pool(name="psKQ", bufs=2, space="PSUM") as psKQ, \
             tc.tile_pool(name="psV", bufs=2, space="PSUM") as psV, \
             tc.tile_pool(name="psA", bufs=1, space="PSUM") as psA:
            wk3 = wt.tile([128, 2 * DOUT], kdt)
            nc.sync.dma_start(out=wk3[:], in_=t_wk3[:])
            wv3 = wt.tile([128, 2 * DOUT], vdt)
            nc.sync.dma_start(out=wv3[:], in_=t_wv3[:])
            wq = wt.tile([128, 2 * DOUT], bf16)
            nc.sync.dma_start(out=wq[:], in_=t_wq[:])
            we = wt.tile([64, DOUT], bf16)
            nc.sync.dma_start(out=we[:], in_=t_we[:])
            wdrow = wt.tile([128, DOUT], bf16)
            nc.sync.dma_start(out=wdrow[:], in_=t_wdrow[:])

            for w in range(NWIN):
                nfe = win.tile([128, tpw * 256], kdt, tag="nfe")
                nc.sync.dma_start(out=nfe[:], in_=t_nfe[w])
                if t_nfe2 is not None:
                    nfe2 = win.tile([128, tpw * 256], vdt, tag="nfe2")
                    nc.sync.dma_start(out=nfe2[:], in_=t_nfe2[w])
                else:
                    nfe2 = nfe
                ohT = win.tile([128, tpw * 128], bf16, tag="ohT")
                nc.scalar.dma_start(out=ohT[:], in_=t_ohT[w])
                ohE = win.tile([128, tpw * 128], bf16, tag="ohE")
                nc.scalar.dma_start(out=ohE[:], in_=t_ohE[w])
                efT = win.tile([64, tpw * 128], bf16, tag="efT")
                nc.sync.dma_start(out=efT[:], in_=t_efT[w])
                nfw = win.tile([128, 256], bf16, tag="nfw")
                nc.sync.dma_start(out=nfw[:], in_=t_nfw[w])

                # qwin = nfw^T @ Wq -> SBUF (borrows a v-pool psum slot)
                ps_qw = psV.tile([128, DOUT], f32, tag="v")
                for i in range(2):
                    nc.tensor.matmul(ps_qw[:], nfw[:, i * 128:(i + 1) * 128],
                                     wq[:, i * DOUT:(i + 1) * DOUT],
                                     start=(i == 0), stop=(i == 1))
                qwin = tb.tile([128, DOUT], bf16, tag="qwin")
                nc.scalar.copy(qwin[:], ps_qw[:])

                ps_agg = psA.tile([128, DOUT + 8], f32, tag="agg")
                pend = None  # deferred (wv, agg) of the previous tile
                for t in range(tpw):
                    nfe_t = nfe[:, t * 256:(t + 1) * 256].rearrange(
                        "p (i m) -> p i m", i=2)
                    nfe2_t = nfe2[:, t * 256:(t + 1) * 256].rearrange(
                        "p (i m) -> p i m", i=2)
                    ohT_t = ohT[:, t * 128:(t + 1) * 128]
                    ohE_t = ohE[:, t * 128:(t + 1) * 128]
                    efT_t = efT[:, t * 128:(t + 1) * 128]

                    # kke | qe into one psum tile  [e, hd]
                    ps_kq = psKQ.tile([128, 2 * DOUT], f32, tag="kq")
                    if k_fp8:
                        nc.tensor.matmul(ps_kq[:, DOUT:], nfe_t,
                                         wv3_like_k(wk3), start=True,
                                         stop=False, perf_mode=MM)
                    else:
                        for i in range(2):
                            nc.tensor.matmul(ps_kq[:, DOUT:], nfe_t[:, i, :],
                                             wk3[:, i * DOUT:(i + 1) * DOUT],
                                             start=(i == 0), stop=False)
                    nc.tensor.matmul(ps_kq[:, DOUT:], efT_t, we[:],
                                     start=False, stop=True)
                    nc.tensor.matmul(ps_kq[:, :DOUT], ohT_t, qwin[:],
                                     start=True, stop=True)
                    kke_sb = tb.tile([128, DOUT], bf16, tag="kke_sb")
                    nc.scalar.copy(kke_sb[:], ps_kq[:, DOUT:])

                    # deferred wv+agg of previous tile (gives exp time to land)
                    if pend is not None:
                        emit_wv_agg(nc, pend, ps_agg)

                    prod = tb.tile([128, DOUT], bf16, tag="prod")
                    nc.vector.tensor_tensor(prod[:], ps_kq[:, :DOUT],
                                            kke_sb[:], mybir.AluOpType.mult)
                    logit = tb.tile([128, 8], f32, tag="logit")
                    nc.vector.tensor_reduce(
                        logit[:], prod[:].rearrange("p (h d) -> p h d", h=H),
                        mybir.AxisListType.X, mybir.AluOpType.add)

                    ps_v = psV.tile([128, DOUT], f32, tag="v")
                    if v_fp8:
                        nc.tensor.matmul(ps_v[:], nfe2_t, wv3[:].rearrange(
                            "p (i n) -> p i n", i=2), start=True, stop=True,
                            perf_mode=MM)
                    else:
                        for i in range(2):
                            nc.tensor.matmul(ps_v[:], nfe2_t[:, i, :],
                                             wv3[:, i * DOUT:(i + 1) * DOUT],
                                             start=(i == 0), stop=(i == 1))

                    v_hd = tb.tile([128, DOUT], bf16, tag="v_sb")
                    nc.scalar.copy(v_hd[:], ps_v[:])
                    wv_ext = tb.tile([128, DOUT + 8], bf16, tag="wv_ext")
                    nc.scalar.activation(wv_ext[:, DOUT:], logit[:],
                                         mybir.ActivationFunctionType.Exp)
                    pend = (wv_ext, v_hd, ohE_t, t == 0, t == tpw - 1)
                emit_wv_agg(nc, pend, ps_agg)

                # ---- window finalize ----
                den = tb.tile([128, 8], f32, tag="den")
                nc.vector.tensor_scalar_add(den[:], ps_agg[:, DOUT:], 1e-9)
                recip = tb.tile([128, 8], f32, tag="recip")
                nc.vector.reciprocal(recip[:], den[:])
                u = tb.tile([128, DOUT], bf16, tag="u")
                nc.vector.scalar_tensor_tensor(
                    out=u[:], in0=ps_agg[:, :DOUT], scalar=0.0,
                    in1=wdrow[:], op0=mybir.AluOpType.max,
                    op1=mybir.AluOpType.mult)
                zscr = tb.tile([128, DOUT], bf16, tag="zscr")
                zacc = tb.tile([128, 1], f32, tag="zacc")
                nc.vector.scalar_tensor_tensor(
                    out=zscr[:].rearrange("p (h d) -> p h d", h=H),
                    in0=u[:].rearrange("p (h d) -> p h d", h=H),
                    scalar=0.0,
                    in1=recip[:, :, None].to_broadcast([128, H, DH]),
                    op0=mybir.AluOpType.add, op1=mybir.AluOpType.mult,
                    accum_out=zacc[:])
                ez = tb.tile([128, 1], f32, tag="ez")
                nc.scalar.activation(ez[:], zacc[:],
                                     mybir.ActivationFunctionType.Exp,
                                     scale=-1.0, bias=-float(bd0))
                ez1 = tb.tile([128, 1], f32, tag="ez1")
                nc.vector.tensor_scalar_add(ez1[:], ez[:], 1.0)
                y_sb = tb.tile([128, 1], f32, tag="y_sb")
                nc.vector.reciprocal(y_sb[:], ez1[:])
                nc.sync.dma_start(out=t_y[w], in_=y_sb[:])
    nc.compile()
    return nc


def wv3_like_k(wk3):
    return wk3[:].rearrange("p (i n) -> p i n", i=2)


def emit_wv_agg(nc, pend, ps_agg):
    wv_ext, v_hd, ohE_t, is_first, is_last = pend
    nc.gpsimd.tensor_tensor(
        wv_ext[:, :DOUT].rearrange("p (h d) -> p h d", h=H),
        wv_ext[:, DOUT:, None].to_broadcast([128, H, DH]),
        v_hd[:].rearrange("p (h d) -> p h d", h=H),
        mybir.AluOpType.mult)
    nc.tensor.matmul(ps_agg[:, :DOUT], ohE_t, wv_ext[:, :DOUT],
                     start=is_first, stop=is_last)
    nc.tensor.matmul(ps_agg[:, DOUT:], ohE_t, wv_ext[:, DOUT:],
                     start=is_first, stop=is_last)


# ----------------------------------------------------------------------------
# Entry point
# ----------------------------------------------------------------------------

LAST_RESULTS = None
LAST_NC = None


def prepare(node_features, edge_features, Wq, Wk, Wv, We, Wd, bd, src, dst,
            dbg=False):
    nf = np.asarray(node_features, dtype=np.float32)
    ef = np.asarray(edge_features, dtype=np.float32)
    src = np.asarray(src, dtype=np.int32)
    dst = np.asarray(dst, dtype=np.int32)
    N = nf.shape[0]

    plan = make_plan(src, dst, N, N_CORES, TPW)
    gin = make_global_inputs(np.asarray(Wq, np.float32),
                             np.asarray(Wk, np.float32),
                             np.asarray(Wv, np.float32),
                             np.asarray(We, np.float32),
                             np.asarray(Wd, np.float32))
    ef_sorted_bf = ef[plan["perm"]].astype(BF16)
    nf_bf = nf.astype(BF16)
    nf_e = nf.astype(FP8) if K_FP8 else nf_bf

    nc = build_nc(NWIN=plan["NWIN"], tpw=TPW, LMAX=plan["LMAX"],
                  bd0=float(np.asarray(bd, np.float32).ravel()[0]),
                  k_fp8=K_FP8, v_fp8=V_FP8, dbg=dbg)

    in_maps = []
    core_meta = []
    for c in range(N_CORES):
        cin = make_core_inputs(plan, c, nf_bf, nf_e, ef_sorted_bf)
        m = dict(gin)
        for k in ("nfe", "ohT", "ohE", "efT", "nfw"):
            m[k] = cin[k]
        if "nfe2" in cin:
            m["nfe2"] = cin["nfe2"]
        in_maps.append(m)
        core_meta.append(plan["cores"][c]["wins"])
    return nc, in_maps, core_meta, N


def kernel(node_features, edge_features, Wq, Wk, Wv, We, Wd, bd, src, dst,
           trace=False, dbg=False, n_cores=None):
    from concourse.bass_utils import run_bass_kernel_spmd

    nc, in_maps, core_meta, N = prepare(node_features, edge_features, Wq, Wk,
                                        Wv, We, Wd, bd, src, dst, dbg=dbg)
    ncores = n_cores or N_CORES
    res = run_bass_kernel_spmd(nc, in_maps[:ncores],
                               core_ids=list(range(ncores)), trace=trace)
    global LAST_RESULTS, LAST_NC
    LAST_RESULTS = res
    LAST_NC = nc

    y = np.zeros((N, 1), np.float32)
    for c, wins in enumerate(core_meta[:ncores]):
        yw = res.results[c]["y_out"]
        for w, (wn_lo, wn_hi) in enumerate(wins):
            y[wn_lo:wn_hi, 0] = yw[w, : wn_hi - wn_lo, 0]
    return y
